# revision 14
# baseline (speedup 1.0000x reference)
"""Cross-attention Trainium2 Bass kernel.

Math (per batch element b, one per NeuronCore):
    q = x Wq + bq            [Sq, 8]
    k = ctx Wk + bk          [Sk, 8]
    v = ctx Wv + bv          [Sk, 8]
    scores = q k^T           [Sq, Sk]
    w = softmax(scores)      (no max subtraction; scores are bounded ~|31|)
    out = w v                [Sq, 8]
    y = out Wo + bo          [Sq, 1024]

Device layout strategy (all contractions need the contracted dim on SBUF
partitions, so x/ctx are fed pre-transposed as xT/ctxT [1024, 2048]):
    qT[8, s]  = Wq^T xT      (PE, stationary = Wq chunk, moving = xT chunk)
    kT[8, t], vT[8, t] from ctxT
    v_ext[t, 33] = PE-transpose of [vT; ones; 0...; ones] (ones at rows 8, 32)
    E^T[t, s] = exp(kT^T qT)             (scoresT via PE, exp via ScalarE)
    oe[33, s] = v_ext^T E^T  accumulated over t-chunks:
        rows 0-7 = sum_t E v   rows 8, 32 = sum_t E  (softmax denominator)
    y[s, d]  = [out; den]^T [Wo; bo]  then scaled by 1/den per-partition.

Matmul operands use float32r (1 PE row/cycle vs 4 for float32, ~13 mantissa
bits). denomT (K=1) and the v_ext transposes run in plain f32 via bitcast
(fp32r unsupported there).
"""

import numpy as np

B = 8
SQ = 2048
SK = 2048
D = 1024
H = 8
N_CORES = 8

_CACHE: dict = {}

_F32R_MBITS = 11


def _round_f32r(x):
    """Round float32 to the PE's fp32r grid (11 mantissa bits, RTN).

    The PE truncates unrounded fp32r operands with ~2x the error of
    round-to-nearest; pre-rounding on the host makes the matmul bit-exact
    w.r.t. the rounded values (measured)."""
    b = np.ascontiguousarray(np.asarray(x, np.float32)).view(np.uint32)
    shift = 23 - _F32R_MBITS
    rb = np.uint32(1 << (shift - 1))
    mask = np.uint32(~((1 << shift) - 1) & 0xFFFFFFFF)
    return ((b + rb) & mask).view(np.float32)


def _build_nc(use_f32r=True):
    import concourse.bacc as bacc
    import concourse.mybir as mybir
    from concourse.bass import ds, ts
    from concourse.tile import TileContext

    F32 = mybir.dt.float32
    MMT = mybir.dt.float32r if use_f32r else F32
    EXP = mybir.ActivationFunctionType.Exp

    nc = bacc.Bacc("TRN2", target_bir_lowering=False, debug=False)

    xT = nc.dram_tensor("xT", [D, SQ], MMT, kind="ExternalInput").ap()
    ctxT = nc.dram_tensor("ctxT", [D, SK], MMT, kind="ExternalInput").ap()
    wq_d = nc.dram_tensor("wq_l", [128, 64], MMT, kind="ExternalInput").ap()
    wkv_d = nc.dram_tensor("wkv_l", [128, 128], MMT, kind="ExternalInput").ap()
    bq_d = nc.dram_tensor("bq8", [8, 1], F32, kind="ExternalInput").ap()
    bk_d = nc.dram_tensor("bk8", [8, 1], F32, kind="ExternalInput").ap()
    bv_d = nc.dram_tensor("bv8", [8, 1], F32, kind="ExternalInput").ap()
    wob_d = nc.dram_tensor("wob", [9, D], MMT, kind="ExternalInput").ap()
    id_d = nc.dram_tensor("ident33", [33, 33], F32, kind="ExternalInput").ap()
    vc_d = nc.dram_tensor("vconst", [25, SK], MMT, kind="ExternalInput").ap()
    on_d = nc.dram_tensor("ones33", [33, 1], MMT, kind="ExternalInput").ap()
    y_d = nc.dram_tensor("y", [SQ, D], F32, kind="ExternalOutput").ap()

    with TileContext(nc) as tc:
        with tc.tile_pool(name="consts", bufs=1) as cp:
            wq_sb = cp.tile([128, 64], MMT)
            wkv_sb = cp.tile([128, 128], MMT)
            bq_sb = cp.tile([8, 1], F32)
            bk_sb = cp.tile([8, 1], F32)
            bv_sb = cp.tile([8, 1], F32)
            wob_sb = cp.tile([9, D], MMT)
            id_sb = cp.tile([33, 33], F32)
            ones_sb = cp.tile([33, 1], MMT)
            nc.sync.dma_start(wq_sb, wq_d)
            nc.sync.dma_start(wkv_sb, wkv_d)
            nc.sync.dma_start(bq_sb, bq_d)
            nc.sync.dma_start(bk_sb, bk_d)
            nc.sync.dma_start(bv_sb, bv_d)
            nc.sync.dma_start(wob_sb, wob_d)
            nc.sync.dma_start(id_sb, id_d)
            nc.sync.dma_start(ones_sb, on_d)

            # persistent activations
            kT_sb = cp.tile([8, SK], MMT)
            qT_sb = cp.tile([8, SQ], MMT)
            vT1_sb = cp.tile([33, SK], MMT)   # v rows 0-7, ones rows 8 & 32
            vext_sb = cp.tile([128, 33 * 16], MMT)
            outU_sb = cp.tile([33, SQ], MMT)  # rows 0-7 outU, 8/32 denom
            rden_sb = cp.tile([128, 16], F32)

            # rows 8..32 of vT1 are constants (ones at 8 and 32, zeros
            # between) — engine writes need 32-aligned partition bases, so
            # fill them via DMA instead of memset.
            nc.sync.dma_start(vT1_sb[8:33, :], vc_d)

            # ---- phase 1: kT / vT projections + v_ext transposes ----
            with tc.tile_pool(name="pin", bufs=2) as inp, \
                 tc.tile_pool(name="pkv", bufs=2, space="PSUM") as pkv, \
                 tc.tile_pool(name="ptr", bufs=2, space="PSUM") as ptr:
                for tt in range(4):
                    ctx_t = inp.tile([128, 4096], MMT, tag="ctx")
                    nc.sync.dma_start(
                        ctx_t.rearrange("p (c s) -> p c s", c=8),
                        ctxT[:, ts(tt, 512)].rearrange("(c p) s -> p c s", p=128),
                    )
                    v_ps = pkv.tile([8, 512], F32, tag="v")
                    k_ps = pkv.tile([8, 512], F32, tag="k")
                    for dc in range(8):
                        nc.tensor.matmul(
                            v_ps, wkv_sb[:, ds(16 * dc, 8)],
                            ctx_t[:, ts(dc, 512)],
                            start=(dc == 0), stop=(dc == 7),
                        )
                        nc.tensor.matmul(
                            k_ps, wkv_sb[:, ds(16 * dc + 8, 8)],
                            ctx_t[:, ts(dc, 512)],
                            start=(dc == 0), stop=(dc == 7),
                        )
                    nc.scalar.add(vT1_sb[0:8, ts(tt, 512)], v_ps, bv_sb[:, 0:1])
                    nc.scalar.add(kT_sb[0:8, ts(tt, 512)], k_ps, bk_sb[:, 0:1])
                    for c in range(4):
                        cc = 4 * tt + c
                        tr_ps = ptr.tile([128, 33], F32)
                        nc.tensor.transpose(
                            tr_ps, vT1_sb[0:33, ts(cc, 128)].bitcast(F32), id_sb)
                        nc.vector.tensor_copy(vext_sb[:, ds(33 * cc, 33)], tr_ps)

            # ---- phase 2: qT projection + attention, per s-tile ----
            with tc.tile_pool(name="pxin", bufs=2) as xinp, \
                 tc.tile_pool(name="pq", bufs=1, space="PSUM") as pq, \
                 tc.tile_pool(name="psc", bufs=2, space="PSUM") as psc, \
                 tc.tile_pool(name="poe", bufs=1, space="PSUM") as poe, \
                 tc.tile_pool(name="pyp", bufs=2, space="PSUM") as pyp, \
                 tc.tile_pool(name="pet", bufs=3) as etp, \
                 tc.tile_pool(name="pys", bufs=2) as ysp:
                for st in range(4):
                    x_t = xinp.tile([128, 4096], MMT, tag="xt")
                    nc.sync.dma_start(
                        x_t.rearrange("p (c s) -> p c s", c=8),
                        xT[:, ts(st, 512)].rearrange("(c p) s -> p c s", p=128),
                    )
                    q_ps = pq.tile([8, 512], F32)
                    for dc in range(8):
                        nc.tensor.matmul(
                            q_ps, wq_sb[:, ds(8 * dc, 8)],
                            x_t[:, ts(dc, 512)],
                            start=(dc == 0), stop=(dc == 7),
                        )
                    nc.scalar.add(qT_sb[0:8, ts(st, 512)], q_ps, bq_sb[:, 0:1])

                    oe_ps = poe.tile([33, 512], F32)
                    for e in range(8):
                        sc_ps = psc.tile([128, 1024], F32)
                        et = etp.tile([128, 1024], MMT, tag="et")
                        for half in range(2):
                            tcn = 2 * e + half
                            nc.tensor.matmul(
                                sc_ps[:, ts(half, 512)],
                                kT_sb[0:8, ts(tcn, 128)],
                                qT_sb[0:8, ts(st, 512)],
                                start=True, stop=True,
                            )
                        nc.scalar.activation(et, sc_ps, EXP)
                        for half in range(2):
                            tcn = 2 * e + half
                            nc.tensor.matmul(
                                oe_ps, vext_sb[:, ds(33 * tcn, 33)],
                                et[:, ts(half, 512)],
                                start=(tcn == 0), stop=(tcn == 15),
                            )
                    nc.scalar.copy(outU_sb[0:33, ts(st, 512)], oe_ps[0:33, :])

                    dp = pyp.tile([128, 4], F32, tag="y")
                    for j in range(4):
                        nc.tensor.matmul(
                            dp[:, ds(j, 1)],
                            outU_sb[32:33, ds(512 * st + 128 * j, 128)].bitcast(F32),
                            ones_sb[32:33, 0:1].bitcast(F32),
                            start=True, stop=True,
                        )
                    nc.vector.reciprocal(rden_sb[:, ts(st, 4)], dp[:, 0:4])

                    for j in range(4):
                        y_sb = ysp.tile([128, 1024], F32, tag="ys")
                        for dh in range(2):
                            y_ps = pyp.tile([128, 512], F32, tag="y")
                            nc.tensor.matmul(
                                y_ps,
                                outU_sb[0:9, ds(512 * st + 128 * j, 128)],
                                wob_sb[:, ts(dh, 512)],
                                start=True, stop=True,
                            )
                            nc.vector.tensor_scalar_mul(
                                y_sb[:, ts(dh, 512)], y_ps,
                                rden_sb[:, ds(4 * st + j, 1)],
                            )
                        nc.sync.dma_start(
                            y_d[ds(128 * (4 * st + j), 128), :], y_sb,
                        )

    nc.compile()
    return nc


def _get_nc(use_f32r=True):
    key = ("nc", use_f32r)
    if key not in _CACHE:
        _CACHE[key] = _build_nc(use_f32r)
    return _CACHE[key]


def _prep_params(Wq, bq, Wk, bk, Wv, bv, Wo, bo):
    f32 = np.float32
    Wq = np.asarray(Wq, f32)
    Wk = np.asarray(Wk, f32)
    Wv = np.asarray(Wv, f32)
    Wo = np.asarray(Wo, f32)
    wq_l = _round_f32r(np.ascontiguousarray(
        Wq.reshape(8, 128, 8).transpose(1, 0, 2).reshape(128, 64)))
    wkv = np.concatenate([Wv, Wk], axis=1)  # v cols 0-7, k cols 8-15
    wkv_l = _round_f32r(np.ascontiguousarray(
        wkv.reshape(8, 128, 16).transpose(1, 0, 2).reshape(128, 128)))
    wob = _round_f32r(np.concatenate(
        [Wo, np.asarray(bo, f32)[None, :]], axis=0))
    ident = np.eye(33, dtype=f32)
    vconst = np.zeros((25, SK), f32)
    vconst[0, :] = 1.0   # vT1 row 8: denominator ones column
    vconst[24, :] = 1.0  # vT1 row 32: denominator copy for denomT matmul
    return {
        "wq_l": wq_l, "wkv_l": wkv_l,
        "bq8": np.asarray(bq, f32).reshape(8, 1),
        "bk8": np.asarray(bk, f32).reshape(8, 1),
        "bv8": np.asarray(bv, f32).reshape(8, 1),
        "wob": np.ascontiguousarray(wob), "ident33": ident,
        "vconst": vconst, "ones33": np.ones((33, 1), f32),
    }


def make_in_maps(x, context, Wq, bq, Wk, bk, Wv, bv, Wo, bo):
    f32 = np.float32
    x = np.asarray(x, f32)
    context = np.asarray(context, f32)
    xT = _round_f32r(np.ascontiguousarray(x.transpose(0, 2, 1)))  # [B, D, SQ]
    ctxT = _round_f32r(np.ascontiguousarray(context.transpose(0, 2, 1)))
    params = _prep_params(Wq, bq, Wk, bk, Wv, bv, Wo, bo)
    return [
        {"xT": xT[b], "ctxT": ctxT[b], **params} for b in range(N_CORES)
    ]


def kernel(x, context, Wq, bq, Wk, bk, Wv, bv, Wo, bo):
    import concourse.bass_utils as bass_utils

    nc = _get_nc()
    in_maps = make_in_maps(x, context, Wq, bq, Wk, bk, Wv, bv, Wo, bo)
    res = bass_utils.run_bass_kernel_spmd(
        nc, in_maps, core_ids=list(range(N_CORES)))
    return np.stack([res.results[b]["y"] for b in range(N_CORES)], axis=0)
